# revision 11
# baseline (speedup 1.0000x reference)
"""AngleSoftmax (SphereFace A-Softmax, m=4, lambda=1000) on 8 trn2 NeuronCores.

Strategy: classic classification-head model parallelism. The class dim C=100000
is sharded 12500/core (padded to 12544 = 98*128). Each core computes its slab
of feat.T = (W_shard @ x.T) * rsqrt(rownorm2(W_shard))  -- [C_loc, B] -- plus a
[512] vector of corrected label-position values computed from the gathered
label rows weight[y] (replicated; pure data movement on host). The host
concatenates slabs, transposes, and scatters the 512 corrected values into
(b, y_b). No collectives are needed.

Per-core device pipeline (all math on device):
  - wT shard arrives host-pre-transposed [2,128,CL] (d-major) so matmul lhsT
    tiles stream straight from DRAM (no on-chip transpose).
  - per-class norm^2 via PE: ones-matmul over DVE-squared wT tiles, landing
    [128,1] columns in PSUM, batched 14 tiles/chunk -> one ACT sqrt + DVE
    reciprocal + one Newton rsqrt refinement step (kills ACT-sqrt table error).
  - main matmul: out[c_tile(128), B=512] = sum over 2 d-halves, f32, PSUM.
  - normalization folded into the mandatory PSUM->SBUF copy: ACT Copy with
    per-partition scale = rsqrt(norm2).
  - label correction: cos at label from x . weight[y] row-wise (DVE
    tensor_tensor_reduce), Chebyshev cos(4t), k = #{cos <= cos(j*pi/4)},
    sign via alternating indicator sum. All [128,4]-shaped (b = 4p+j).
"""

import sys

if "/opt/trn_rl_repo" not in sys.path:
    sys.path.insert(0, "/opt/trn_rl_repo")

import numpy as np

B, D, C, NCORES = 512, 256, 100000, 8
CPC = C // NCORES          # 12500 real classes per core
CL = 12544                 # 98 * 128, padded
CHUNK = 1792               # 14 c-tiles per chunk, 7 chunks
NT = CHUNK // 128          # 14
LAMB = 1000.0

_NC = None


def _build_nc(with_corr=True, pair_out=False, with_norm=True, corr_level=5, use_ttr=False):
    from contextlib import ExitStack

    import concourse.bacc as bacc
    import concourse.mybir as mybir
    from concourse import tile

    f32 = mybir.dt.float32
    AF = mybir.ActivationFunctionType
    OP = mybir.AluOpType

    nc = bacc.Bacc("TRN2", target_bir_lowering=False, debug=False,
                   num_devices=NCORES)

    wT = nc.declare_dram_parameter("wT", [2, 128, CL], f32, isOutput=False)
    xTd = nc.declare_dram_parameter("xT", [2, 128, B], f32, isOutput=False)
    x4d = nc.declare_dram_parameter("x4", [128, 4 * D], f32, isOutput=False)
    wg4d = nc.declare_dram_parameter("wg4", [128, 4 * D], f32, isOutput=False)
    outd = nc.declare_dram_parameter("out", [CL, B], f32, isOutput=True)
    corrd = nc.declare_dram_parameter("corr", [128, 4], f32, isOutput=True)

    with tile.TileContext(nc) as tc, ExitStack() as ctx:
        cons = ctx.enter_context(tc.tile_pool(name="cons", bufs=1))
        wtp = ctx.enter_context(tc.tile_pool(name="wtp", bufs=3))
        wt2p = ctx.enter_context(tc.tile_pool(name="wt2p", bufs=2))
        obp = ctx.enter_context(tc.tile_pool(name="obp", bufs=4))
        smp = ctx.enter_context(tc.tile_pool(name="smp", bufs=2))
        rp = ctx.enter_context(tc.tile_pool(name="rp", bufs=3))
        pso = ctx.enter_context(tc.tile_pool(name="pso", bufs=4, space="PSUM"))
        psn = ctx.enter_context(tc.tile_pool(name="psn", bufs=2, space="PSUM"))

        ones = cons.tile([128, 1], f32)
        nc.vector.memset(ones[:], 1.0)
        xt0 = cons.tile([128, B], f32)
        nc.sync.dma_start(out=xt0[:], in_=xTd[0])
        xt1 = cons.tile([128, B], f32)
        nc.sync.dma_start(out=xt1[:], in_=xTd[1])

        def rsqrt_refined(n2_ap, w, key):
            # r = rsqrt(n2), Newton-refined once so ACT Sqrt's loose table
            # error (<=65536 ULP budget) cannot leak into the output.
            s = rp.tile([128, w], f32, tag=f"s_{key}")
            nc.scalar.activation(s[:], n2_ap, AF.Sqrt)
            r0 = rp.tile([128, w], f32, tag=f"r0_{key}")
            nc.vector.reciprocal(r0[:], s[:])
            q = rp.tile([128, w], f32, tag=f"q_{key}")
            nc.vector.tensor_mul(q[:], r0[:], r0[:])
            nc.vector.tensor_mul(q[:], q[:], n2_ap)
            nc.vector.tensor_scalar(out=q[:], in0=q[:], scalar1=-0.5,
                                    scalar2=1.5, op0=OP.mult, op1=OP.add)
            r = rp.tile([128, w], f32, tag=f"r_{key}")
            nc.vector.tensor_mul(r[:], r0[:], q[:])
            return r

        # ---------- label-position correction (tiny, [128,4]) ----------
        if with_corr:
            x4s = cons.tile([128, 4 * D], f32)
            nc.sync.dma_start(out=x4s[:], in_=x4d[:])
            wg4s = cons.tile([128, 4 * D], f32)
            nc.sync.dma_start(out=wg4s[:], in_=wg4d[:])

            xl2 = cons.tile([128, 4], f32)
            gn2 = cons.tile([128, 4], f32)
            dg = cons.tile([128, 4], f32)
            for j in range(4):
                xj = x4s[:, j * D:(j + 1) * D]
                gj = wg4s[:, j * D:(j + 1) * D]
                for in0, in1, acc in ((xj, xj, xl2), (gj, gj, gn2),
                                      (xj, gj, dg)):
                    scr = smp.tile([128, D], f32, tag="scr")
                    if use_ttr:
                        nc.vector.tensor_tensor_reduce(
                            out=scr[:], in0=in0, in1=in1, scale=1.0,
                            scalar=0.0, op0=OP.mult, op1=OP.add,
                            accum_out=acc[:, j:j + 1])
                    else:
                        nc.vector.tensor_mul(scr[:], in0, in1)
                        nc.vector.tensor_reduce(
                            acc[:, j:j + 1], scr[:], mybir.AxisListType.X,
                            OP.add)

            if corr_level <= 1:
                nc.sync.dma_start(out=corrd[:], in_=dg[:])
            if corr_level >= 2:
                rxl = rsqrt_refined(xl2[:], 4, "xl")
                rgs = rsqrt_refined(gn2[:], 4, "gs")
                xlen = cons.tile([128, 4], f32)
                nc.vector.tensor_mul(xlen[:], xl2[:], rxl[:])
                featl = cons.tile([128, 4], f32)
                nc.vector.tensor_mul(featl[:], dg[:], rgs[:])   # cos_l * xlen
                cosl = cons.tile([128, 4], f32)
                nc.vector.tensor_mul(cosl[:], featl[:], rxl[:])
                if corr_level == 2:
                    nc.sync.dma_start(out=corrd[:], in_=cosl[:])
            if corr_level >= 3:
                c2 = cons.tile([128, 4], f32)
                nc.vector.tensor_mul(c2[:], cosl[:], cosl[:])
                c4 = cons.tile([128, 4], f32)
                nc.vector.tensor_mul(c4[:], c2[:], c2[:])
                cos4 = cons.tile([128, 4], f32)
                nc.vector.tensor_sub(cos4[:], c4[:], c2[:])
                nc.vector.tensor_scalar(out=cos4[:], in0=cos4[:], scalar1=8.0,
                                        scalar2=1.0, op0=OP.mult, op1=OP.add)
                if corr_level == 3:
                    nc.sync.dma_start(out=corrd[:], in_=cos4[:])
            if corr_level >= 4:
                # k = floor(4*arccos(c)/pi) = #{j in 1..4 : c <= cos(j*pi/4)}
                thr = [float(np.sqrt(0.5)), 0.0, float(-np.sqrt(0.5)), -1.0]
                a = []
                for i, th in enumerate(thr):
                    ai = cons.tile([128, 4], f32, tag=f"a{i}")
                    nc.vector.tensor_scalar(out=ai[:], in0=cosl[:],
                                            scalar1=th, scalar2=None,
                                            op0=OP.is_le)
                    a.append(ai)
                k = cons.tile([128, 4], f32)
                nc.vector.tensor_add(k[:], a[0][:], a[1][:])
                nc.vector.tensor_add(k[:], k[:], a[2][:])
                nc.vector.tensor_add(k[:], k[:], a[3][:])
                # (-1)^k: monotone indicators, k mod 2 = a1-a2+a3-a4
                alt = cons.tile([128, 4], f32)
                nc.vector.tensor_sub(alt[:], a[0][:], a[1][:])
                alt2 = cons.tile([128, 4], f32)
                nc.vector.tensor_sub(alt2[:], a[2][:], a[3][:])
                nc.vector.tensor_add(alt[:], alt[:], alt2[:])
                sgn = cons.tile([128, 4], f32)
                nc.vector.tensor_scalar(out=sgn[:], in0=alt[:], scalar1=-2.0,
                                        scalar2=1.0, op0=OP.mult, op1=OP.add)
                if corr_level == 4:
                    nc.sync.dma_start(out=corrd[:], in_=sgn[:])
            if corr_level >= 5:
                phi = cons.tile([128, 4], f32)
                nc.vector.tensor_mul(phi[:], sgn[:], cos4[:])
                km2 = cons.tile([128, 4], f32)
                nc.vector.tensor_scalar(out=km2[:], in0=k[:], scalar1=-2.0,
                                        scalar2=None, op0=OP.mult)
                nc.vector.tensor_add(phi[:], phi[:], km2[:])
                # corr = (LAMB*featl + phi*xlen) / (1+LAMB)
                px = cons.tile([128, 4], f32)
                nc.vector.tensor_mul(px[:], phi[:], xlen[:])
                nc.vector.tensor_scalar(out=px[:], in0=px[:],
                                        scalar1=1.0 / (1.0 + LAMB),
                                        scalar2=None, op0=OP.mult)
                fl2 = cons.tile([128, 4], f32)
                nc.vector.tensor_scalar(out=fl2[:], in0=featl[:],
                                        scalar1=LAMB / (1.0 + LAMB),
                                        scalar2=None, op0=OP.mult)
                corr_sb = cons.tile([128, 4], f32)
                nc.vector.tensor_add(corr_sb[:], px[:], fl2[:])
                nc.sync.dma_start(out=corrd[:], in_=corr_sb[:])
        else:
            zc = cons.tile([128, 4], f32)
            nc.vector.memset(zc[:], 0.0)
            nc.sync.dma_start(out=corrd[:], in_=zc[:])

        # ---------- main slab ----------
        for ci in range(CL // CHUNK):
            c0 = ci * CHUNK
            wt0 = wtp.tile([128, CHUNK], f32, tag="wt0")
            nc.sync.dma_start(out=wt0[:], in_=wT[0, :, c0:c0 + CHUNK])
            wt1 = wtp.tile([128, CHUNK], f32, tag="wt1")
            nc.sync.dma_start(out=wt1[:], in_=wT[1, :, c0:c0 + CHUNK])

            if with_norm:
                w20 = wt2p.tile([128, CHUNK], f32, tag="w20")
                nc.vector.tensor_mul(w20[:], wt0[:], wt0[:])
                w21 = wt2p.tile([128, CHUNK], f32, tag="w21")
                nc.vector.tensor_mul(w21[:], wt1[:], wt1[:])
                pn = psn.tile([128, 16], f32, tag="pn")
                for t in range(NT):
                    nc.tensor.matmul(pn[:, t:t + 1],
                                     w20[:, 128 * t:128 * (t + 1)],
                                     ones[:], start=True, stop=False)
                    nc.tensor.matmul(pn[:, t:t + 1],
                                     w21[:, 128 * t:128 * (t + 1)],
                                     ones[:], start=False, stop=True)
                r = rsqrt_refined(pn[:, 0:NT], NT, "n")

            step = 2 if pair_out else 1
            for t0 in range(0, NT, step):
                num = min(step, NT - t0)
                ob = obp.tile([128, B * step], f32, tag="ob")
                for u in range(num):
                    t = t0 + u
                    po = pso.tile([128, B], f32, tag="po")
                    nc.tensor.matmul(po[:], wt0[:, 128 * t:128 * (t + 1)],
                                     xt0[:], start=True, stop=False)
                    nc.tensor.matmul(po[:], wt1[:, 128 * t:128 * (t + 1)],
                                     xt1[:], start=False, stop=True)
                    if with_norm:
                        nc.scalar.mul(ob[:, B * u:B * (u + 1)], po[:],
                                      r[:, t:t + 1])
                    else:
                        nc.scalar.mul(ob[:, B * u:B * (u + 1)], po[:], 1.0)
                if num == 1:
                    nc.sync.dma_start(
                        out=outd[c0 + 128 * t0: c0 + 128 * (t0 + 1), :],
                        in_=ob[:, 0:B])
                else:
                    dst = outd[c0 + 128 * t0: c0 + 128 * (t0 + num), :]
                    dst = dst.rearrange("(u p) b -> p u b", p=128)
                    src = ob[:, 0:B * num].rearrange("p (u b) -> p u b", u=num)
                    nc.sync.dma_start(out=dst, in_=src)

    nc.compile()
    return nc


def get_nc(**opts):
    global _NC
    if _NC is None:
        _NC = _build_nc(**opts)
    return _NC


def make_in_maps(x, weight, y):
    x = np.ascontiguousarray(np.asarray(x, dtype=np.float32))
    weight = np.ascontiguousarray(np.asarray(weight, dtype=np.float32))
    yi = np.asarray(y).astype(np.int64)

    xT = np.ascontiguousarray(x.T).reshape(2, 128, B)
    x4 = np.ascontiguousarray(x.reshape(128, 4 * D))
    wg4 = np.ascontiguousarray(weight[yi].reshape(128, 4 * D))

    in_maps = []
    for m in range(NCORES):
        shard = np.ones((CL, D), dtype=np.float32)
        shard[:CPC] = weight[m * CPC:(m + 1) * CPC]
        wTm = np.ascontiguousarray(shard.T).reshape(2, 128, CL)
        in_maps.append({"wT": wTm, "xT": xT, "x4": x4, "wg4": wg4})
    return in_maps, yi


def run_spmd(in_maps, **kwargs):
    from concourse.bass_utils import run_bass_kernel_spmd

    nc = get_nc()
    return run_bass_kernel_spmd(nc, in_maps, core_ids=list(range(NCORES)),
                                **kwargs)


def assemble(results, yi):
    out = np.empty((B, C), dtype=np.float32)
    for m in range(NCORES):
        out[:, m * CPC:(m + 1) * CPC] = results[m]["out"][:CPC, :].T
    corr = np.asarray(results[0]["corr"], dtype=np.float32).reshape(B)
    out[np.arange(B), yi] = corr
    return out


def kernel(x=None, weight=None, y=None, **_unused):
    in_maps, yi = make_in_maps(x, weight, y)
    res = run_spmd(in_maps)
    return assemble(res.results, yi)


if __name__ == "__main__":
    nc = get_nc()
    print("built ok")


# revision 12
# speedup vs baseline: 1.7406x; 1.7406x over previous
"""AngleSoftmax (SphereFace A-Softmax, m=4, lambda=1000) on 8 trn2 NeuronCores.

Strategy: classic classification-head model parallelism. The class dim C=100000
is sharded 12500/core (padded to 12544 = 98*128). Each core computes its slab
of feat.T = (W_shard @ x.T) * rsqrt(rownorm2(W_shard))  -- [C_loc, B] -- plus a
[512] vector of corrected label-position values computed from the gathered
label rows weight[y] (replicated; pure data movement on host). The host
concatenates slabs, transposes, and scatters the 512 corrected values into
(b, y_b). No collectives are needed.

Per-core device pipeline (all math on device):
  - wT shard arrives host-pre-transposed [2,128,CL] (d-major) so matmul lhsT
    tiles stream straight from DRAM (no on-chip transpose).
  - per-class norm^2 via PE: ones-matmul over DVE-squared wT tiles, landing
    [128,1] columns in PSUM, batched 14 tiles/chunk -> one ACT sqrt + DVE
    reciprocal + one Newton rsqrt refinement step (kills ACT-sqrt table error).
  - main matmul: out[c_tile(128), B=512] = sum over 2 d-halves, f32, PSUM.
  - normalization folded into the mandatory PSUM->SBUF copy: ACT Copy with
    per-partition scale = rsqrt(norm2).
  - label correction: cos at label from x . weight[y] row-wise (DVE
    tensor_tensor_reduce), Chebyshev cos(4t), k = #{cos <= cos(j*pi/4)},
    sign via alternating indicator sum. All [128,4]-shaped (b = 4p+j).
"""

import sys

if "/opt/trn_rl_repo" not in sys.path:
    sys.path.insert(0, "/opt/trn_rl_repo")

import numpy as np

B, D, C, NCORES = 512, 256, 100000, 8
CPC = C // NCORES          # 12500 real classes per core
CL = 12544                 # 98 * 128, padded
CHUNK = 1792               # 14 c-tiles per chunk, 7 chunks
NT = CHUNK // 128          # 14
LAMB = 1000.0

_NC = None


def _build_nc(with_corr=True, pair_out=False, with_norm=True, corr_level=5, use_ttr=False):
    from contextlib import ExitStack

    import concourse.bacc as bacc
    import concourse.mybir as mybir
    from concourse import tile

    f32 = mybir.dt.float32
    bf16 = mybir.dt.bfloat16
    AF = mybir.ActivationFunctionType
    OP = mybir.AluOpType

    nc = bacc.Bacc("TRN2", target_bir_lowering=False, debug=False,
                   num_devices=NCORES)

    wT = nc.declare_dram_parameter("wT", [2, 128, CL], bf16, isOutput=False)
    xTd = nc.declare_dram_parameter("xT", [2, 128, B], bf16, isOutput=False)
    x4d = nc.declare_dram_parameter("x4", [128, 4 * D], f32, isOutput=False)
    wg4d = nc.declare_dram_parameter("wg4", [128, 4 * D], f32, isOutput=False)
    outd = nc.declare_dram_parameter("out", [CL, B], f32, isOutput=True)
    corrd = nc.declare_dram_parameter("corr", [128, 4], f32, isOutput=True)

    with tile.TileContext(nc) as tc, ExitStack() as ctx:
        cons = ctx.enter_context(tc.tile_pool(name="cons", bufs=1))
        wtp = ctx.enter_context(tc.tile_pool(name="wtp", bufs=3))
        wt2p = ctx.enter_context(tc.tile_pool(name="wt2p", bufs=2))
        obp = ctx.enter_context(tc.tile_pool(name="obp", bufs=4))
        smp = ctx.enter_context(tc.tile_pool(name="smp", bufs=2))
        rp = ctx.enter_context(tc.tile_pool(name="rp", bufs=3))
        pso = ctx.enter_context(tc.tile_pool(name="pso", bufs=4, space="PSUM"))
        psn = ctx.enter_context(tc.tile_pool(name="psn", bufs=2, space="PSUM"))

        ones = cons.tile([128, 1], bf16)
        nc.vector.memset(ones[:], 1.0)
        xt0 = cons.tile([128, B], bf16)
        nc.sync.dma_start(out=xt0[:], in_=xTd[0])
        xt1 = cons.tile([128, B], bf16)
        nc.sync.dma_start(out=xt1[:], in_=xTd[1])

        def rsqrt_refined(n2_ap, w, key):
            # r = rsqrt(n2), Newton-refined once so ACT Sqrt's loose table
            # error (<=65536 ULP budget) cannot leak into the output.
            s = rp.tile([128, w], f32, tag=f"s_{key}")
            nc.scalar.activation(s[:], n2_ap, AF.Sqrt)
            r0 = rp.tile([128, w], f32, tag=f"r0_{key}")
            nc.vector.reciprocal(r0[:], s[:])
            q = rp.tile([128, w], f32, tag=f"q_{key}")
            nc.vector.tensor_mul(q[:], r0[:], r0[:])
            nc.vector.tensor_mul(q[:], q[:], n2_ap)
            nc.vector.tensor_scalar(out=q[:], in0=q[:], scalar1=-0.5,
                                    scalar2=1.5, op0=OP.mult, op1=OP.add)
            r = rp.tile([128, w], f32, tag=f"r_{key}")
            nc.vector.tensor_mul(r[:], r0[:], q[:])
            return r

        # ---------- label-position correction (tiny, [128,4]) ----------
        if with_corr:
            x4s = cons.tile([128, 4 * D], f32)
            nc.sync.dma_start(out=x4s[:], in_=x4d[:])
            wg4s = cons.tile([128, 4 * D], f32)
            nc.sync.dma_start(out=wg4s[:], in_=wg4d[:])

            xl2 = cons.tile([128, 4], f32)
            gn2 = cons.tile([128, 4], f32)
            dg = cons.tile([128, 4], f32)
            for j in range(4):
                xj = x4s[:, j * D:(j + 1) * D]
                gj = wg4s[:, j * D:(j + 1) * D]
                for in0, in1, acc in ((xj, xj, xl2), (gj, gj, gn2),
                                      (xj, gj, dg)):
                    scr = smp.tile([128, D], f32, tag="scr")
                    if use_ttr:
                        nc.vector.tensor_tensor_reduce(
                            out=scr[:], in0=in0, in1=in1, scale=1.0,
                            scalar=0.0, op0=OP.mult, op1=OP.add,
                            accum_out=acc[:, j:j + 1])
                    else:
                        nc.vector.tensor_mul(scr[:], in0, in1)
                        nc.vector.tensor_reduce(
                            acc[:, j:j + 1], scr[:], mybir.AxisListType.X,
                            OP.add)

            if corr_level <= 1:
                nc.sync.dma_start(out=corrd[:], in_=dg[:])
            if corr_level >= 2:
                rxl = rsqrt_refined(xl2[:], 4, "xl")
                rgs = rsqrt_refined(gn2[:], 4, "gs")
                xlen = cons.tile([128, 4], f32)
                nc.vector.tensor_mul(xlen[:], xl2[:], rxl[:])
                featl = cons.tile([128, 4], f32)
                nc.vector.tensor_mul(featl[:], dg[:], rgs[:])   # cos_l * xlen
                cosl = cons.tile([128, 4], f32)
                nc.vector.tensor_mul(cosl[:], featl[:], rxl[:])
                if corr_level == 2:
                    nc.sync.dma_start(out=corrd[:], in_=cosl[:])
            if corr_level >= 3:
                c2 = cons.tile([128, 4], f32)
                nc.vector.tensor_mul(c2[:], cosl[:], cosl[:])
                c4 = cons.tile([128, 4], f32)
                nc.vector.tensor_mul(c4[:], c2[:], c2[:])
                cos4 = cons.tile([128, 4], f32)
                nc.vector.tensor_sub(cos4[:], c4[:], c2[:])
                nc.vector.tensor_scalar(out=cos4[:], in0=cos4[:], scalar1=8.0,
                                        scalar2=1.0, op0=OP.mult, op1=OP.add)
                if corr_level == 3:
                    nc.sync.dma_start(out=corrd[:], in_=cos4[:])
            if corr_level >= 4:
                # k = floor(4*arccos(c)/pi) = #{j in 1..4 : c <= cos(j*pi/4)}
                thr = [float(np.sqrt(0.5)), 0.0, float(-np.sqrt(0.5)), -1.0]
                a = []
                for i, th in enumerate(thr):
                    ai = cons.tile([128, 4], f32, tag=f"a{i}")
                    nc.vector.tensor_scalar(out=ai[:], in0=cosl[:],
                                            scalar1=th, scalar2=None,
                                            op0=OP.is_le)
                    a.append(ai)
                k = cons.tile([128, 4], f32)
                nc.vector.tensor_add(k[:], a[0][:], a[1][:])
                nc.vector.tensor_add(k[:], k[:], a[2][:])
                nc.vector.tensor_add(k[:], k[:], a[3][:])
                # (-1)^k: monotone indicators, k mod 2 = a1-a2+a3-a4
                alt = cons.tile([128, 4], f32)
                nc.vector.tensor_sub(alt[:], a[0][:], a[1][:])
                alt2 = cons.tile([128, 4], f32)
                nc.vector.tensor_sub(alt2[:], a[2][:], a[3][:])
                nc.vector.tensor_add(alt[:], alt[:], alt2[:])
                sgn = cons.tile([128, 4], f32)
                nc.vector.tensor_scalar(out=sgn[:], in0=alt[:], scalar1=-2.0,
                                        scalar2=1.0, op0=OP.mult, op1=OP.add)
                if corr_level == 4:
                    nc.sync.dma_start(out=corrd[:], in_=sgn[:])
            if corr_level >= 5:
                phi = cons.tile([128, 4], f32)
                nc.vector.tensor_mul(phi[:], sgn[:], cos4[:])
                km2 = cons.tile([128, 4], f32)
                nc.vector.tensor_scalar(out=km2[:], in0=k[:], scalar1=-2.0,
                                        scalar2=None, op0=OP.mult)
                nc.vector.tensor_add(phi[:], phi[:], km2[:])
                # corr = (LAMB*featl + phi*xlen) / (1+LAMB)
                px = cons.tile([128, 4], f32)
                nc.vector.tensor_mul(px[:], phi[:], xlen[:])
                nc.vector.tensor_scalar(out=px[:], in0=px[:],
                                        scalar1=1.0 / (1.0 + LAMB),
                                        scalar2=None, op0=OP.mult)
                fl2 = cons.tile([128, 4], f32)
                nc.vector.tensor_scalar(out=fl2[:], in0=featl[:],
                                        scalar1=LAMB / (1.0 + LAMB),
                                        scalar2=None, op0=OP.mult)
                corr_sb = cons.tile([128, 4], f32)
                nc.vector.tensor_add(corr_sb[:], px[:], fl2[:])
                nc.sync.dma_start(out=corrd[:], in_=corr_sb[:])
        else:
            zc = cons.tile([128, 4], f32)
            nc.vector.memset(zc[:], 0.0)
            nc.sync.dma_start(out=corrd[:], in_=zc[:])

        # ---------- main slab ----------
        for ci in range(CL // CHUNK):
            c0 = ci * CHUNK
            wt0 = wtp.tile([128, CHUNK], bf16, tag="wt0")
            nc.sync.dma_start(out=wt0[:], in_=wT[0, :, c0:c0 + CHUNK])
            wt1 = wtp.tile([128, CHUNK], bf16, tag="wt1")
            nc.sync.dma_start(out=wt1[:], in_=wT[1, :, c0:c0 + CHUNK])

            if with_norm:
                w20 = wt2p.tile([128, CHUNK], bf16, tag="w20")
                nc.vector.tensor_mul(w20[:], wt0[:], wt0[:])
                w21 = wt2p.tile([128, CHUNK], bf16, tag="w21")
                nc.vector.tensor_mul(w21[:], wt1[:], wt1[:])
                pn = psn.tile([128, 16], f32, tag="pn")
                for t in range(NT):
                    nc.tensor.matmul(pn[:, t:t + 1],
                                     w20[:, 128 * t:128 * (t + 1)],
                                     ones[:], start=True, stop=False)
                    nc.tensor.matmul(pn[:, t:t + 1],
                                     w21[:, 128 * t:128 * (t + 1)],
                                     ones[:], start=False, stop=True)
                r = rsqrt_refined(pn[:, 0:NT], NT, "n")

            step = 2 if pair_out else 1
            for t0 in range(0, NT, step):
                num = min(step, NT - t0)
                ob = obp.tile([128, B * step], f32, tag="ob")
                for u in range(num):
                    t = t0 + u
                    po = pso.tile([128, B], f32, tag="po")
                    nc.tensor.matmul(po[:], wt0[:, 128 * t:128 * (t + 1)],
                                     xt0[:], start=True, stop=False)
                    nc.tensor.matmul(po[:], wt1[:, 128 * t:128 * (t + 1)],
                                     xt1[:], start=False, stop=True)
                    if with_norm:
                        if t % 2 == 0:
                            nc.scalar.mul(ob[:, B * u:B * (u + 1)], po[:],
                                          r[:, t:t + 1])
                        else:
                            nc.vector.tensor_scalar(
                                out=ob[:, B * u:B * (u + 1)], in0=po[:],
                                scalar1=r[:, t:t + 1], scalar2=None,
                                op0=OP.mult)
                    else:
                        nc.scalar.mul(ob[:, B * u:B * (u + 1)], po[:], 1.0)
                if num == 1:
                    nc.sync.dma_start(
                        out=outd[c0 + 128 * t0: c0 + 128 * (t0 + 1), :],
                        in_=ob[:, 0:B])
                else:
                    dst = outd[c0 + 128 * t0: c0 + 128 * (t0 + num), :]
                    dst = dst.rearrange("(u p) b -> p u b", p=128)
                    src = ob[:, 0:B * num].rearrange("p (u b) -> p u b", u=num)
                    nc.sync.dma_start(out=dst, in_=src)

    nc.compile()
    return nc


def get_nc(**opts):
    global _NC
    if _NC is None:
        _NC = _build_nc(**opts)
    return _NC


def make_in_maps(x, weight, y):
    x = np.ascontiguousarray(np.asarray(x, dtype=np.float32))
    weight = np.ascontiguousarray(np.asarray(weight, dtype=np.float32))
    yi = np.asarray(y).astype(np.int64)

    import ml_dtypes
    bf = ml_dtypes.bfloat16
    xT = np.ascontiguousarray(x.T.astype(bf)).reshape(2, 128, B)
    x4 = np.ascontiguousarray(x.reshape(128, 4 * D))
    wg4 = np.ascontiguousarray(weight[yi].reshape(128, 4 * D))

    in_maps = []
    for m in range(NCORES):
        shard = np.ones((CL, D), dtype=np.float32)
        shard[:CPC] = weight[m * CPC:(m + 1) * CPC]
        wTm = np.ascontiguousarray(shard.T.astype(bf)).reshape(2, 128, CL)
        in_maps.append({"wT": wTm, "xT": xT, "x4": x4, "wg4": wg4})
    return in_maps, yi


def run_spmd(in_maps, **kwargs):
    from concourse.bass_utils import run_bass_kernel_spmd

    nc = get_nc()
    return run_bass_kernel_spmd(nc, in_maps, core_ids=list(range(NCORES)),
                                **kwargs)


def assemble(results, yi):
    out = np.empty((B, C), dtype=np.float32)
    for m in range(NCORES):
        out[:, m * CPC:(m + 1) * CPC] = results[m]["out"][:CPC, :].T
    corr = np.asarray(results[0]["corr"], dtype=np.float32).reshape(B)
    out[np.arange(B), yi] = corr
    return out


def kernel(x=None, weight=None, y=None, **_unused):
    in_maps, yi = make_in_maps(x, weight, y)
    res = run_spmd(in_maps)
    return assemble(res.results, yi)


if __name__ == "__main__":
    nc = get_nc()
    print("built ok")


# revision 13
# speedup vs baseline: 2.0988x; 1.2058x over previous
"""AngleSoftmax (SphereFace A-Softmax, m=4, lambda=1000) on 8 trn2 NeuronCores.

Strategy: classic classification-head model parallelism. The class dim C=100000
is sharded 12500/core (padded to 12544 = 98*128). Each core computes its slab
of feat.T = (W_shard @ x.T) * rsqrt(rownorm2(W_shard))  -- [C_loc, B] -- plus a
[512] vector of corrected label-position values computed from the gathered
label rows weight[y] (replicated; pure data movement on host). The host
concatenates slabs, transposes, and scatters the 512 corrected values into
(b, y_b). No collectives are needed.

Per-core device pipeline (all math on device):
  - wT shard arrives host-pre-transposed [2,128,CL] (d-major) so matmul lhsT
    tiles stream straight from DRAM (no on-chip transpose).
  - per-class norm^2 via PE: ones-matmul over DVE-squared wT tiles, landing
    [128,1] columns in PSUM, batched 14 tiles/chunk -> one ACT sqrt + DVE
    reciprocal + one Newton rsqrt refinement step (kills ACT-sqrt table error).
  - main matmul: out[c_tile(128), B=512] = sum over 2 d-halves, f32, PSUM.
  - normalization folded into the mandatory PSUM->SBUF copy: ACT Copy with
    per-partition scale = rsqrt(norm2).
  - label correction: cos at label from x . weight[y] row-wise (DVE
    tensor_tensor_reduce), Chebyshev cos(4t), k = #{cos <= cos(j*pi/4)},
    sign via alternating indicator sum. All [128,4]-shaped (b = 4p+j).
"""

import sys

if "/opt/trn_rl_repo" not in sys.path:
    sys.path.insert(0, "/opt/trn_rl_repo")

import numpy as np

B, D, C, NCORES = 512, 256, 100000, 8
CPC = C // NCORES          # 12500 real classes per core
CL = 12544                 # 98 * 128, padded
CHUNK = 1792               # 14 c-tiles per chunk, 7 chunks
NT = CHUNK // 128          # 14
LAMB = 1000.0

_NC = None


def _build_nc(with_corr=True, pair_out=False, with_norm=True, corr_level=5, use_ttr=False):
    from contextlib import ExitStack

    import concourse.bacc as bacc
    import concourse.mybir as mybir
    from concourse import tile

    f32 = mybir.dt.float32
    bf16 = mybir.dt.bfloat16
    AF = mybir.ActivationFunctionType
    OP = mybir.AluOpType

    nc = bacc.Bacc("TRN2", target_bir_lowering=False, debug=False,
                   num_devices=NCORES)

    wT = nc.declare_dram_parameter("wT", [2, 128, CL], bf16, isOutput=False)
    xTd = nc.declare_dram_parameter("xT", [2, 128, B], bf16, isOutput=False)
    x4d = nc.declare_dram_parameter("x4", [128, 4 * D], f32, isOutput=False)
    wg4d = nc.declare_dram_parameter("wg4", [128, 4 * D], f32, isOutput=False)
    outd = nc.declare_dram_parameter("out", [CL, B], bf16, isOutput=True)
    corrd = nc.declare_dram_parameter("corr", [128, 4], f32, isOutput=True)

    with tile.TileContext(nc) as tc, ExitStack() as ctx:
        cons = ctx.enter_context(tc.tile_pool(name="cons", bufs=1))
        wtp = ctx.enter_context(tc.tile_pool(name="wtp", bufs=3))
        wt2p = ctx.enter_context(tc.tile_pool(name="wt2p", bufs=2))
        obp = ctx.enter_context(tc.tile_pool(name="obp", bufs=4))
        smp = ctx.enter_context(tc.tile_pool(name="smp", bufs=2))
        rp = ctx.enter_context(tc.tile_pool(name="rp", bufs=3))
        pso = ctx.enter_context(tc.tile_pool(name="pso", bufs=4, space="PSUM"))
        psn = ctx.enter_context(tc.tile_pool(name="psn", bufs=2, space="PSUM"))

        ones = cons.tile([128, 1], bf16)
        nc.vector.memset(ones[:], 1.0)
        xt0 = cons.tile([128, B], bf16)
        nc.sync.dma_start(out=xt0[:], in_=xTd[0])
        xt1 = cons.tile([128, B], bf16)
        nc.sync.dma_start(out=xt1[:], in_=xTd[1])

        def rsqrt_refined(n2_ap, w, key):
            # r = rsqrt(n2), Newton-refined once so ACT Sqrt's loose table
            # error (<=65536 ULP budget) cannot leak into the output.
            s = rp.tile([128, w], f32, tag=f"s_{key}")
            nc.scalar.activation(s[:], n2_ap, AF.Sqrt)
            r0 = rp.tile([128, w], f32, tag=f"r0_{key}")
            nc.vector.reciprocal(r0[:], s[:])
            q = rp.tile([128, w], f32, tag=f"q_{key}")
            nc.vector.tensor_mul(q[:], r0[:], r0[:])
            nc.vector.tensor_mul(q[:], q[:], n2_ap)
            nc.vector.tensor_scalar(out=q[:], in0=q[:], scalar1=-0.5,
                                    scalar2=1.5, op0=OP.mult, op1=OP.add)
            r = rp.tile([128, w], f32, tag=f"r_{key}")
            nc.vector.tensor_mul(r[:], r0[:], q[:])
            return r

        # ---------- label-position correction (tiny, [128,4]) ----------
        if with_corr:
            x4s = cons.tile([128, 4 * D], f32)
            nc.sync.dma_start(out=x4s[:], in_=x4d[:])
            wg4s = cons.tile([128, 4 * D], f32)
            nc.sync.dma_start(out=wg4s[:], in_=wg4d[:])

            xl2 = cons.tile([128, 4], f32)
            gn2 = cons.tile([128, 4], f32)
            dg = cons.tile([128, 4], f32)
            for j in range(4):
                xj = x4s[:, j * D:(j + 1) * D]
                gj = wg4s[:, j * D:(j + 1) * D]
                for in0, in1, acc in ((xj, xj, xl2), (gj, gj, gn2),
                                      (xj, gj, dg)):
                    scr = smp.tile([128, D], f32, tag="scr")
                    if use_ttr:
                        nc.vector.tensor_tensor_reduce(
                            out=scr[:], in0=in0, in1=in1, scale=1.0,
                            scalar=0.0, op0=OP.mult, op1=OP.add,
                            accum_out=acc[:, j:j + 1])
                    else:
                        nc.vector.tensor_mul(scr[:], in0, in1)
                        nc.vector.tensor_reduce(
                            acc[:, j:j + 1], scr[:], mybir.AxisListType.X,
                            OP.add)

            if corr_level <= 1:
                nc.sync.dma_start(out=corrd[:], in_=dg[:])
            if corr_level >= 2:
                rxl = rsqrt_refined(xl2[:], 4, "xl")
                rgs = rsqrt_refined(gn2[:], 4, "gs")
                xlen = cons.tile([128, 4], f32)
                nc.vector.tensor_mul(xlen[:], xl2[:], rxl[:])
                featl = cons.tile([128, 4], f32)
                nc.vector.tensor_mul(featl[:], dg[:], rgs[:])   # cos_l * xlen
                cosl = cons.tile([128, 4], f32)
                nc.vector.tensor_mul(cosl[:], featl[:], rxl[:])
                if corr_level == 2:
                    nc.sync.dma_start(out=corrd[:], in_=cosl[:])
            if corr_level >= 3:
                c2 = cons.tile([128, 4], f32)
                nc.vector.tensor_mul(c2[:], cosl[:], cosl[:])
                c4 = cons.tile([128, 4], f32)
                nc.vector.tensor_mul(c4[:], c2[:], c2[:])
                cos4 = cons.tile([128, 4], f32)
                nc.vector.tensor_sub(cos4[:], c4[:], c2[:])
                nc.vector.tensor_scalar(out=cos4[:], in0=cos4[:], scalar1=8.0,
                                        scalar2=1.0, op0=OP.mult, op1=OP.add)
                if corr_level == 3:
                    nc.sync.dma_start(out=corrd[:], in_=cos4[:])
            if corr_level >= 4:
                # k = floor(4*arccos(c)/pi) = #{j in 1..4 : c <= cos(j*pi/4)}
                thr = [float(np.sqrt(0.5)), 0.0, float(-np.sqrt(0.5)), -1.0]
                a = []
                for i, th in enumerate(thr):
                    ai = cons.tile([128, 4], f32, tag=f"a{i}")
                    nc.vector.tensor_scalar(out=ai[:], in0=cosl[:],
                                            scalar1=th, scalar2=None,
                                            op0=OP.is_le)
                    a.append(ai)
                k = cons.tile([128, 4], f32)
                nc.vector.tensor_add(k[:], a[0][:], a[1][:])
                nc.vector.tensor_add(k[:], k[:], a[2][:])
                nc.vector.tensor_add(k[:], k[:], a[3][:])
                # (-1)^k: monotone indicators, k mod 2 = a1-a2+a3-a4
                alt = cons.tile([128, 4], f32)
                nc.vector.tensor_sub(alt[:], a[0][:], a[1][:])
                alt2 = cons.tile([128, 4], f32)
                nc.vector.tensor_sub(alt2[:], a[2][:], a[3][:])
                nc.vector.tensor_add(alt[:], alt[:], alt2[:])
                sgn = cons.tile([128, 4], f32)
                nc.vector.tensor_scalar(out=sgn[:], in0=alt[:], scalar1=-2.0,
                                        scalar2=1.0, op0=OP.mult, op1=OP.add)
                if corr_level == 4:
                    nc.sync.dma_start(out=corrd[:], in_=sgn[:])
            if corr_level >= 5:
                phi = cons.tile([128, 4], f32)
                nc.vector.tensor_mul(phi[:], sgn[:], cos4[:])
                km2 = cons.tile([128, 4], f32)
                nc.vector.tensor_scalar(out=km2[:], in0=k[:], scalar1=-2.0,
                                        scalar2=None, op0=OP.mult)
                nc.vector.tensor_add(phi[:], phi[:], km2[:])
                # corr = (LAMB*featl + phi*xlen) / (1+LAMB)
                px = cons.tile([128, 4], f32)
                nc.vector.tensor_mul(px[:], phi[:], xlen[:])
                nc.vector.tensor_scalar(out=px[:], in0=px[:],
                                        scalar1=1.0 / (1.0 + LAMB),
                                        scalar2=None, op0=OP.mult)
                fl2 = cons.tile([128, 4], f32)
                nc.vector.tensor_scalar(out=fl2[:], in0=featl[:],
                                        scalar1=LAMB / (1.0 + LAMB),
                                        scalar2=None, op0=OP.mult)
                corr_sb = cons.tile([128, 4], f32)
                nc.vector.tensor_add(corr_sb[:], px[:], fl2[:])
                nc.sync.dma_start(out=corrd[:], in_=corr_sb[:])
        else:
            zc = cons.tile([128, 4], f32)
            nc.vector.memset(zc[:], 0.0)
            nc.sync.dma_start(out=corrd[:], in_=zc[:])

        # ---------- main slab ----------
        for ci in range(CL // CHUNK):
            c0 = ci * CHUNK
            wt0 = wtp.tile([128, CHUNK], bf16, tag="wt0")
            nc.sync.dma_start(out=wt0[:], in_=wT[0, :, c0:c0 + CHUNK])
            wt1 = wtp.tile([128, CHUNK], bf16, tag="wt1")
            nc.sync.dma_start(out=wt1[:], in_=wT[1, :, c0:c0 + CHUNK])

            if with_norm:
                w20 = wt2p.tile([128, CHUNK], bf16, tag="w20")
                nc.vector.tensor_mul(w20[:], wt0[:], wt0[:])
                w21 = wt2p.tile([128, CHUNK], bf16, tag="w21")
                nc.vector.tensor_mul(w21[:], wt1[:], wt1[:])
                pn = psn.tile([128, 16], f32, tag="pn")
                for t in range(NT):
                    nc.tensor.matmul(pn[:, t:t + 1],
                                     w20[:, 128 * t:128 * (t + 1)],
                                     ones[:], start=True, stop=False)
                    nc.tensor.matmul(pn[:, t:t + 1],
                                     w21[:, 128 * t:128 * (t + 1)],
                                     ones[:], start=False, stop=True)
                r = rsqrt_refined(pn[:, 0:NT], NT, "n")

            step = 2 if pair_out else 1
            for t0 in range(0, NT, step):
                num = min(step, NT - t0)
                ob = obp.tile([128, B * step], bf16, tag="ob")
                for u in range(num):
                    t = t0 + u
                    po = pso.tile([128, B], f32, tag="po")
                    nc.tensor.matmul(po[:], wt0[:, 128 * t:128 * (t + 1)],
                                     xt0[:], start=True, stop=False)
                    nc.tensor.matmul(po[:], wt1[:, 128 * t:128 * (t + 1)],
                                     xt1[:], start=False, stop=True)
                    if with_norm:
                        if t % 2 == 0:
                            nc.scalar.mul(ob[:, B * u:B * (u + 1)], po[:],
                                          r[:, t:t + 1])
                        else:
                            nc.vector.tensor_scalar(
                                out=ob[:, B * u:B * (u + 1)], in0=po[:],
                                scalar1=r[:, t:t + 1], scalar2=None,
                                op0=OP.mult)
                    else:
                        nc.scalar.mul(ob[:, B * u:B * (u + 1)], po[:], 1.0)
                if num == 1:
                    nc.sync.dma_start(
                        out=outd[c0 + 128 * t0: c0 + 128 * (t0 + 1), :],
                        in_=ob[:, 0:B])
                else:
                    dst = outd[c0 + 128 * t0: c0 + 128 * (t0 + num), :]
                    dst = dst.rearrange("(u p) b -> p u b", p=128)
                    src = ob[:, 0:B * num].rearrange("p (u b) -> p u b", u=num)
                    nc.sync.dma_start(out=dst, in_=src)

    nc.compile()
    return nc


def get_nc(**opts):
    global _NC
    if _NC is None:
        _NC = _build_nc(**opts)
    return _NC


def make_in_maps(x, weight, y):
    x = np.ascontiguousarray(np.asarray(x, dtype=np.float32))
    weight = np.ascontiguousarray(np.asarray(weight, dtype=np.float32))
    yi = np.asarray(y).astype(np.int64)

    import ml_dtypes
    bf = ml_dtypes.bfloat16
    xT = np.ascontiguousarray(x.T.astype(bf)).reshape(2, 128, B)
    x4 = np.ascontiguousarray(x.reshape(128, 4 * D))
    wg4 = np.ascontiguousarray(weight[yi].reshape(128, 4 * D))

    in_maps = []
    for m in range(NCORES):
        shard = np.ones((CL, D), dtype=np.float32)
        shard[:CPC] = weight[m * CPC:(m + 1) * CPC]
        wTm = np.ascontiguousarray(shard.T.astype(bf)).reshape(2, 128, CL)
        in_maps.append({"wT": wTm, "xT": xT, "x4": x4, "wg4": wg4})
    return in_maps, yi


def run_spmd(in_maps, **kwargs):
    from concourse.bass_utils import run_bass_kernel_spmd

    nc = get_nc()
    return run_bass_kernel_spmd(nc, in_maps, core_ids=list(range(NCORES)),
                                **kwargs)


def assemble(results, yi):
    out = np.empty((B, C), dtype=np.float32)
    for m in range(NCORES):
        out[:, m * CPC:(m + 1) * CPC] = (
            results[m]["out"][:CPC, :].astype(np.float32).T)
    corr = np.asarray(results[0]["corr"], dtype=np.float32).reshape(B)
    out[np.arange(B), yi] = corr
    return out


def kernel(x=None, weight=None, y=None, **_unused):
    in_maps, yi = make_in_maps(x, weight, y)
    res = run_spmd(in_maps)
    return assemble(res.results, yi)


if __name__ == "__main__":
    nc = get_nc()
    print("built ok")


# revision 14
# speedup vs baseline: 2.2149x; 1.0553x over previous
"""AngleSoftmax (SphereFace A-Softmax, m=4, lambda=1000) on 8 trn2 NeuronCores.

Strategy: classic classification-head model parallelism. The class dim C=100000
is sharded 12500/core (padded to 12544 = 98*128). Each core computes its slab
of feat.T = (W_shard @ x.T) * rsqrt(rownorm2(W_shard))  -- [C_loc, B] -- plus a
[512] vector of corrected label-position values computed from the gathered
label rows weight[y] (replicated; pure data movement on host). The host
concatenates slabs, transposes, and scatters the 512 corrected values into
(b, y_b). No collectives are needed.

Per-core device pipeline (all math on device):
  - wT shard arrives host-pre-transposed [2,128,CL] (d-major) so matmul lhsT
    tiles stream straight from DRAM (no on-chip transpose).
  - per-class norm^2 via PE: ones-matmul over DVE-squared wT tiles, landing
    [128,1] columns in PSUM, batched 14 tiles/chunk -> one ACT sqrt + DVE
    reciprocal + one Newton rsqrt refinement step (kills ACT-sqrt table error).
  - main matmul: out[c_tile(128), B=512] = sum over 2 d-halves, f32, PSUM.
  - normalization folded into the mandatory PSUM->SBUF copy: ACT Copy with
    per-partition scale = rsqrt(norm2).
  - label correction: cos at label from x . weight[y] row-wise (DVE
    tensor_tensor_reduce), Chebyshev cos(4t), k = #{cos <= cos(j*pi/4)},
    sign via alternating indicator sum. All [128,4]-shaped (b = 4p+j).
"""

import sys

if "/opt/trn_rl_repo" not in sys.path:
    sys.path.insert(0, "/opt/trn_rl_repo")

import numpy as np

B, D, C, NCORES = 512, 256, 100000, 8
CPC = C // NCORES          # 12500 real classes per core
CL = 12544                 # 98 * 128, padded
CHUNK = 1792               # 14 c-tiles per chunk, 7 chunks
NT = CHUNK // 128          # 14
LAMB = 1000.0

_NC = None


def _build_nc(with_corr=True, pair_out=False, with_norm=True, corr_level=5, use_ttr=False):
    from contextlib import ExitStack

    import concourse.bacc as bacc
    import concourse.mybir as mybir
    from concourse import tile

    f32 = mybir.dt.float32
    bf16 = mybir.dt.bfloat16
    AF = mybir.ActivationFunctionType
    OP = mybir.AluOpType

    nc = bacc.Bacc("TRN2", target_bir_lowering=False, debug=False,
                   num_devices=NCORES)

    wT = nc.declare_dram_parameter("wT", [2, 128, CL], bf16, isOutput=False)
    xTd = nc.declare_dram_parameter("xT", [2, 128, B], bf16, isOutput=False)
    x4d = nc.declare_dram_parameter("x4", [128, 4 * D], f32, isOutput=False)
    wg4d = nc.declare_dram_parameter("wg4", [128, 4 * D], f32, isOutput=False)
    outd = nc.declare_dram_parameter("out", [128, (CL // 128) * B], bf16, isOutput=True)
    corrd = nc.declare_dram_parameter("corr", [128, 4], f32, isOutput=True)

    with tile.TileContext(nc) as tc, ExitStack() as ctx:
        cons = ctx.enter_context(tc.tile_pool(name="cons", bufs=1))
        wtp = ctx.enter_context(tc.tile_pool(name="wtp", bufs=3))
        wt2p = ctx.enter_context(tc.tile_pool(name="wt2p", bufs=2))
        obp = ctx.enter_context(tc.tile_pool(name="obp", bufs=4))
        smp = ctx.enter_context(tc.tile_pool(name="smp", bufs=2))
        rp = ctx.enter_context(tc.tile_pool(name="rp", bufs=3))
        pso = ctx.enter_context(tc.tile_pool(name="pso", bufs=4, space="PSUM"))
        psn = ctx.enter_context(tc.tile_pool(name="psn", bufs=2, space="PSUM"))

        ones = cons.tile([128, 1], bf16)
        nc.vector.memset(ones[:], 1.0)
        xt0 = cons.tile([128, B], bf16)
        nc.sync.dma_start(out=xt0[:], in_=xTd[0])
        xt1 = cons.tile([128, B], bf16)
        nc.sync.dma_start(out=xt1[:], in_=xTd[1])

        def rsqrt_refined(n2_ap, w, key):
            # r = rsqrt(n2), Newton-refined once so ACT Sqrt's loose table
            # error (<=65536 ULP budget) cannot leak into the output.
            s = rp.tile([128, w], f32, tag=f"s_{key}")
            nc.scalar.activation(s[:], n2_ap, AF.Sqrt)
            r0 = rp.tile([128, w], f32, tag=f"r0_{key}")
            nc.vector.reciprocal(r0[:], s[:])
            q = rp.tile([128, w], f32, tag=f"q_{key}")
            nc.vector.tensor_mul(q[:], r0[:], r0[:])
            nc.vector.tensor_mul(q[:], q[:], n2_ap)
            nc.vector.tensor_scalar(out=q[:], in0=q[:], scalar1=-0.5,
                                    scalar2=1.5, op0=OP.mult, op1=OP.add)
            r = rp.tile([128, w], f32, tag=f"r_{key}")
            nc.vector.tensor_mul(r[:], r0[:], q[:])
            return r

        # ---------- label-position correction (tiny, [128,4]) ----------
        if with_corr:
            x4s = cons.tile([128, 4 * D], f32)
            nc.sync.dma_start(out=x4s[:], in_=x4d[:])
            wg4s = cons.tile([128, 4 * D], f32)
            nc.sync.dma_start(out=wg4s[:], in_=wg4d[:])

            xl2 = cons.tile([128, 4], f32)
            gn2 = cons.tile([128, 4], f32)
            dg = cons.tile([128, 4], f32)
            for j in range(4):
                xj = x4s[:, j * D:(j + 1) * D]
                gj = wg4s[:, j * D:(j + 1) * D]
                for in0, in1, acc in ((xj, xj, xl2), (gj, gj, gn2),
                                      (xj, gj, dg)):
                    scr = smp.tile([128, D], f32, tag="scr")
                    if use_ttr:
                        nc.vector.tensor_tensor_reduce(
                            out=scr[:], in0=in0, in1=in1, scale=1.0,
                            scalar=0.0, op0=OP.mult, op1=OP.add,
                            accum_out=acc[:, j:j + 1])
                    else:
                        nc.vector.tensor_mul(scr[:], in0, in1)
                        nc.vector.tensor_reduce(
                            acc[:, j:j + 1], scr[:], mybir.AxisListType.X,
                            OP.add)

            if corr_level <= 1:
                nc.sync.dma_start(out=corrd[:], in_=dg[:])
            if corr_level >= 2:
                rxl = rsqrt_refined(xl2[:], 4, "xl")
                rgs = rsqrt_refined(gn2[:], 4, "gs")
                xlen = cons.tile([128, 4], f32)
                nc.vector.tensor_mul(xlen[:], xl2[:], rxl[:])
                featl = cons.tile([128, 4], f32)
                nc.vector.tensor_mul(featl[:], dg[:], rgs[:])   # cos_l * xlen
                cosl = cons.tile([128, 4], f32)
                nc.vector.tensor_mul(cosl[:], featl[:], rxl[:])
                if corr_level == 2:
                    nc.sync.dma_start(out=corrd[:], in_=cosl[:])
            if corr_level >= 3:
                c2 = cons.tile([128, 4], f32)
                nc.vector.tensor_mul(c2[:], cosl[:], cosl[:])
                c4 = cons.tile([128, 4], f32)
                nc.vector.tensor_mul(c4[:], c2[:], c2[:])
                cos4 = cons.tile([128, 4], f32)
                nc.vector.tensor_sub(cos4[:], c4[:], c2[:])
                nc.vector.tensor_scalar(out=cos4[:], in0=cos4[:], scalar1=8.0,
                                        scalar2=1.0, op0=OP.mult, op1=OP.add)
                if corr_level == 3:
                    nc.sync.dma_start(out=corrd[:], in_=cos4[:])
            if corr_level >= 4:
                # k = floor(4*arccos(c)/pi) = #{j in 1..4 : c <= cos(j*pi/4)}
                thr = [float(np.sqrt(0.5)), 0.0, float(-np.sqrt(0.5)), -1.0]
                a = []
                for i, th in enumerate(thr):
                    ai = cons.tile([128, 4], f32, tag=f"a{i}")
                    nc.vector.tensor_scalar(out=ai[:], in0=cosl[:],
                                            scalar1=th, scalar2=None,
                                            op0=OP.is_le)
                    a.append(ai)
                k = cons.tile([128, 4], f32)
                nc.vector.tensor_add(k[:], a[0][:], a[1][:])
                nc.vector.tensor_add(k[:], k[:], a[2][:])
                nc.vector.tensor_add(k[:], k[:], a[3][:])
                # (-1)^k: monotone indicators, k mod 2 = a1-a2+a3-a4
                alt = cons.tile([128, 4], f32)
                nc.vector.tensor_sub(alt[:], a[0][:], a[1][:])
                alt2 = cons.tile([128, 4], f32)
                nc.vector.tensor_sub(alt2[:], a[2][:], a[3][:])
                nc.vector.tensor_add(alt[:], alt[:], alt2[:])
                sgn = cons.tile([128, 4], f32)
                nc.vector.tensor_scalar(out=sgn[:], in0=alt[:], scalar1=-2.0,
                                        scalar2=1.0, op0=OP.mult, op1=OP.add)
                if corr_level == 4:
                    nc.sync.dma_start(out=corrd[:], in_=sgn[:])
            if corr_level >= 5:
                phi = cons.tile([128, 4], f32)
                nc.vector.tensor_mul(phi[:], sgn[:], cos4[:])
                km2 = cons.tile([128, 4], f32)
                nc.vector.tensor_scalar(out=km2[:], in0=k[:], scalar1=-2.0,
                                        scalar2=None, op0=OP.mult)
                nc.vector.tensor_add(phi[:], phi[:], km2[:])
                # corr = (LAMB*featl + phi*xlen) / (1+LAMB)
                px = cons.tile([128, 4], f32)
                nc.vector.tensor_mul(px[:], phi[:], xlen[:])
                nc.vector.tensor_scalar(out=px[:], in0=px[:],
                                        scalar1=1.0 / (1.0 + LAMB),
                                        scalar2=None, op0=OP.mult)
                fl2 = cons.tile([128, 4], f32)
                nc.vector.tensor_scalar(out=fl2[:], in0=featl[:],
                                        scalar1=LAMB / (1.0 + LAMB),
                                        scalar2=None, op0=OP.mult)
                corr_sb = cons.tile([128, 4], f32)
                nc.vector.tensor_add(corr_sb[:], px[:], fl2[:])
                nc.sync.dma_start(out=corrd[:], in_=corr_sb[:])
        else:
            zc = cons.tile([128, 4], f32)
            nc.vector.memset(zc[:], 0.0)
            nc.sync.dma_start(out=corrd[:], in_=zc[:])

        # ---------- main slab ----------
        for ci in range(CL // CHUNK):
            c0 = ci * CHUNK
            wt0 = wtp.tile([128, CHUNK], bf16, tag="wt0")
            nc.sync.dma_start(out=wt0[:], in_=wT[0, :, c0:c0 + CHUNK])
            wt1 = wtp.tile([128, CHUNK], bf16, tag="wt1")
            nc.sync.dma_start(out=wt1[:], in_=wT[1, :, c0:c0 + CHUNK])

            if with_norm:
                w20 = wt2p.tile([128, CHUNK], bf16, tag="w20")
                nc.vector.tensor_mul(w20[:], wt0[:], wt0[:])
                w21 = wt2p.tile([128, CHUNK], bf16, tag="w21")
                nc.vector.tensor_mul(w21[:], wt1[:], wt1[:])
                pn = psn.tile([128, 16], f32, tag="pn")
                for t in range(NT):
                    nc.tensor.matmul(pn[:, t:t + 1],
                                     w20[:, 128 * t:128 * (t + 1)],
                                     ones[:], start=True, stop=False)
                    nc.tensor.matmul(pn[:, t:t + 1],
                                     w21[:, 128 * t:128 * (t + 1)],
                                     ones[:], start=False, stop=True)
                r = rsqrt_refined(pn[:, 0:NT], NT, "n")

            ob = obp.tile([128, NT * B], bf16, tag="ob")
            for t in range(NT):
                po = pso.tile([128, B], f32, tag="po")
                nc.tensor.matmul(po[:], wt0[:, 128 * t:128 * (t + 1)],
                                 xt0[:], start=True, stop=False)
                nc.tensor.matmul(po[:], wt1[:, 128 * t:128 * (t + 1)],
                                 xt1[:], start=False, stop=True)
                dst = ob[:, B * t:B * (t + 1)]
                if not with_norm:
                    nc.scalar.mul(dst, po[:], 1.0)
                elif t % 2 == 0:
                    nc.scalar.mul(dst, po[:], r[:, t:t + 1])
                else:
                    nc.vector.tensor_scalar(out=dst, in0=po[:],
                                            scalar1=r[:, t:t + 1],
                                            scalar2=None, op0=OP.mult)
            nc.sync.dma_start(out=outd[:, ci * NT * B:(ci + 1) * NT * B],
                              in_=ob[:])

    nc.compile()
    return nc


def get_nc(**opts):
    global _NC
    if _NC is None:
        _NC = _build_nc(**opts)
    return _NC


def make_in_maps(x, weight, y):
    x = np.ascontiguousarray(np.asarray(x, dtype=np.float32))
    weight = np.ascontiguousarray(np.asarray(weight, dtype=np.float32))
    yi = np.asarray(y).astype(np.int64)

    import ml_dtypes
    bf = ml_dtypes.bfloat16
    xT = np.ascontiguousarray(x.T.astype(bf)).reshape(2, 128, B)
    x4 = np.ascontiguousarray(x.reshape(128, 4 * D))
    wg4 = np.ascontiguousarray(weight[yi].reshape(128, 4 * D))

    in_maps = []
    for m in range(NCORES):
        shard = np.ones((CL, D), dtype=np.float32)
        shard[:CPC] = weight[m * CPC:(m + 1) * CPC]
        wTm = np.ascontiguousarray(shard.T.astype(bf)).reshape(2, 128, CL)
        in_maps.append({"wT": wTm, "xT": xT, "x4": x4, "wg4": wg4})
    return in_maps, yi


def run_spmd(in_maps, **kwargs):
    from concourse.bass_utils import run_bass_kernel_spmd

    nc = get_nc()
    return run_bass_kernel_spmd(nc, in_maps, core_ids=list(range(NCORES)),
                                **kwargs)


def assemble(results, yi):
    out = np.empty((B, C), dtype=np.float32)
    for m in range(NCORES):
        arr = np.asarray(results[m]["out"]).reshape(128, CL // 128, B)
        slabT = arr.transpose(1, 0, 2).reshape(CL, B)
        out[:, m * CPC:(m + 1) * CPC] = slabT[:CPC].astype(np.float32).T
    corr = np.asarray(results[0]["corr"], dtype=np.float32).reshape(B)
    out[np.arange(B), yi] = corr
    return out


def kernel(x=None, weight=None, y=None, **_unused):
    in_maps, yi = make_in_maps(x, weight, y)
    res = run_spmd(in_maps)
    return assemble(res.results, yi)


if __name__ == "__main__":
    nc = get_nc()
    print("built ok")
